# revision 12
# baseline (speedup 1.0000x reference)
"""PointConvolution (8-neighbor shifted diffs + 1x1 conv) as a single 3x3 conv,
run data-parallel across 8 TRN2 NeuronCores via Bass/Tile.

Math: out[o,h,w] = sum_k sum_c W[o,3k+c] * (xpad[c,h+ik,w+jk] - x[c,h,w]) + b[o]
    = sum_{c,i,j} K3[o,c,i,j] * xpad[c,h+i,w+j] + b[o]
  where K3 gets W at the 8 non-center taps and -sum(W over taps) at center.

v7 (fp16 I/O; rel-err budget is 2e-2, fp16 keeps us ~1e-3):
  - All HBM traffic in fp16: input im2row blocks and the output. This halves
    the DMA floor vs the fp32 v6 (~214us -> ~120us/core at 358 GB/s).
  - Host sends TWO pre-shifted im2row blocks per chunk (j=0 and j=1); the
    j=1 shift is odd (2 bytes) so an on-chip fp16 copy could only run in 1x
    DVE mode -- from DRAM it rides the (cheap) DMA instead. j=2 is an even
    shift (4B-aligned) so it IS built on-chip at 4x DVE mode.
  - Partition map (K=68): j0 @ 0..17 (DMA), zero gap @ 18..31, j2 @ 32..49
    (DVE copy of j0 shifted left 2 cols; dst/src starts 32-aligned),
    j1 @ 50..67 (DMA). Gap partitions zeroed once per pool buffer at warmup
    (zero weight rows there; zeroing avoids NaN*0=NaN from garbage).
  - Per group t: ONE self-loading fp16 matmul [68x128]x[68x512] (start+stop).
  - PSUM -> SBUF drain adds bias and converts to fp16, split between DVE
    (tensor_scalar_add, cols 0..DW) and ACT (Identity+bias, cols DW..512)
    so both engines land ~85us, under the ~121us DMA floor.
  - One 128-partition fp16 DMA per chunk writes a permuted DRAM layout;
    host transposes + upcasts during unshard.
"""

import numpy as np

import concourse.bacc as bacc
import concourse.bass as bass
import concourse.tile as tile
from concourse import mybir
from concourse.bass_utils import run_bass_kernel_spmd

# Problem constants (hardcoded per harness contract)
B, C, H, W_DIM, OUT = 16, 3, 512, 512, 32
KS, P = 3, 1
NCORES = 8
NB = B // NCORES          # images per core = 2
Hp, Wp = H + 2 * P, W_DIM + 2 * P   # 514, 514

G = 4                     # output rows per matmul group
S = G + KS - 1            # input rows per group window = 6
T = 4                     # groups per PSUM half
TB = 8                    # groups per chunk (32 output rows)
CH = G * TB               # 32 output rows per chunk
NCHUNK = H // CH          # 16 chunks per image
K0 = C * S                # 18 contraction partitions per j-block
GAP = 14                  # zero partitions 18..31 (j2 block must start at 32)
OFF_J = (0, 50, 32)       # partition start of the j=0,1,2 blocks
K = 68                    # total contraction partitions incl. zero gap
M = G * OUT               # 128 output partitions
FW = TB * Wp              # 4112 free cols per contraction row
OBF = 2 * T * W_DIM       # 4096 free cols in the output tile
XBUFS = 8                 # xin pool depth (warmup-zeroed once per buffer)
LA_DMA = 6                # chunks of input-DMA lookahead
LA_J2 = 3                 # chunks of j2-replica lookahead (after its DMA: 3)
T2 = 2                    # groups per PSUM tile (2 banks; 4 tiles in flight)
NQ = TB // T2             # PSUM tiles per chunk
DW = 152                  # drain cols per 512 done on DVE (rest on ACT)

F32 = mybir.dt.float32
F16 = mybir.dt.float16


def _coords():
    i, j = np.meshgrid(np.arange(KS), np.arange(KS))
    coords = np.dstack((i.reshape(-1), j.reshape(-1)))[0]
    return coords[np.any(coords != P, axis=1)]


def _build_weights(W, b):
    K3 = np.zeros((OUT, C, KS, KS), np.float32)
    Wr = W.reshape(OUT, 8, C)
    for k, (i, j) in enumerate(_coords()):
        K3[:, :, i, j] += Wr[:, k, :]
    K3[:, :, P, P] = -Wr.sum(axis=1)

    # wt[OFF_J[j] + Sc + s, 32g + o] = K3[o, c, s-g, j] when 0 <= s-g < KS
    wt = np.zeros((K, M), np.float32)
    for j in range(KS):
        for c in range(C):
            for s in range(S):
                for g in range(G):
                    i = s - g
                    if 0 <= i < KS:
                        wt[OFF_J[j] + S * c + s, OUT * g: OUT * (g + 1)] = K3[:, c, i, j]
    bias = np.tile(b.astype(np.float32), G).reshape(M, 1)
    return wt.astype(np.float16), bias


def _build_xin(x):
    """[B,C,H,W] -> [B, NCHUNK, 2, K0, FW] fp16: j=0 and j=1 im2row blocks."""
    xpad = np.pad(np.ascontiguousarray(x, np.float32),
                  ((0, 0), (0, 0), (P, P), (P, P))).astype(np.float16)
    ch = np.arange(NCHUNK)[:, None, None]
    s = np.arange(S)[None, :, None]
    t = np.arange(TB)[None, None, :]
    rows = CH * ch + G * t + s                      # [NCHUNK, S, TB]
    big = xpad[:, :, rows, :]                       # [B, C, NCHUNK, S, TB, Wp]
    big = big.transpose(0, 2, 1, 3, 4, 5)           # [B, NCHUNK, C, S, TB, Wp]
    j0 = np.ascontiguousarray(big).reshape(B, NCHUNK, K0, FW)
    xin = np.zeros((B, NCHUNK, 2, K0, FW), np.float16)
    xin[:, :, 0] = j0
    xin[:, :, 1, :, :FW - 1] = j0[:, :, :, 1:]
    return xin


def _build_bass():
    # Bacc (not plain Bass): its compile() runs move_matmul_waits_to_ldweights
    # and generate_event_semaphores, required because TRN2 instructions take
    # at most one semaphore wait.
    nc = bacc.Bacc("TRN2")
    x_d = nc.declare_dram_parameter("xin", [NB, NCHUNK, 2, K0, FW], F16, isOutput=False)
    wt_d = nc.declare_dram_parameter("wt", [K, M], F16, isOutput=False)
    b_d = nc.declare_dram_parameter("bias", [M, 1], F32, isOutput=False)
    z_d = nc.declare_dram_parameter("zeros", [GAP, FW], F16, isOutput=False)
    out_d = nc.declare_dram_parameter("out", [NB, NCHUNK, M, OBF], F16, isOutput=True)

    with tile.TileContext(nc) as tc:
        with (
            tc.tile_pool(name="wpool", bufs=1) as wpool,
            tc.tile_pool(name="xpool", bufs=XBUFS) as xpool,
            tc.tile_pool(name="opool", bufs=4) as opool,
            tc.tile_pool(name="psum", bufs=4, space=bass.MemorySpace.PSUM) as ppool,
        ):
            wsb = wpool.tile([K, M], F16)
            nc.scalar.dma_start(wsb[:], wt_d[:])
            bsb = wpool.tile([M, 1], F32)
            nc.scalar.dma_start(bsb[:], b_d[:])

            # Gap partitions (zero weight rows) are zeroed on the first XBUFS
            # loop tiles -- on the SAME tile the matmul reads, so Tile tracks
            # the RAW dep. Later chunks never write gaps, so zeros persist
            # per pool slot.
            zsrc = bass.AP(z_d, 0, [[FW, GAP], [1, FW]])

            def dma_stage(idx):
                """Issue input DMAs for chunk idx; return tile."""
                xin = xpool.tile([K, FW], F16)
                if idx < XBUFS:
                    nc.gpsimd.dma_start(xin[K0:K0 + GAP, :], zsrc)
                base = idx * 2 * K0 * FW
                src0 = bass.AP(x_d, base, [[FW, K0], [1, FW]])
                src1 = bass.AP(x_d, base + K0 * FW, [[FW, K0], [1, FW]])
                nc.gpsimd.dma_start(xin[:K0, :], src0)
                nc.gpsimd.dma_start(xin[50:50 + K0, :], src1)
                return xin

            def j2_stage(xin):
                # j=2 replica: j0 shifted left 2 cols (4B-aligned -> 4x DVE).
                # Issued LA_J2 chunks ahead of compute but LA_DMA-LA_J2 chunks
                # AFTER its input DMA was issued: the DVE queue is in-order,
                # so this op must never reach the queue head while its DMA is
                # still in flight -- it would stall the drains queued behind
                # it (this was the v8/v9 bottleneck).
                nc.vector.tensor_scalar_add(
                    xin[32: 32 + K0, : FW - 2], xin[:K0, 2:], 0.0,
                )

            TOTAL = NB * NCHUNK
            tiles = [dma_stage(i) for i in range(min(LA_DMA, TOTAL))]
            for i in range(min(LA_J2, TOTAL)):
                j2_stage(tiles[i])
            for idx in range(TOTAL):
                if idx + LA_DMA < TOTAL:
                    tiles.append(dma_stage(idx + LA_DMA))
                if idx + LA_J2 < TOTAL:
                    j2_stage(tiles[idx + LA_J2])
                xin = tiles[idx]

                ob = opool.tile([M, NQ, T2, W_DIM], F16)
                for quarter in range(NQ):
                    ps = ppool.tile([M, T2, W_DIM], F32)
                    for t2 in range(T2):
                        t = quarter * T2 + t2
                        nc.tensor.matmul(
                            ps[:, t2, :],
                            wsb[:],
                            xin[:, Wp * t: Wp * t + W_DIM],
                            start=True,
                            stop=True,
                        )
                    # PSUM drain + bias + fp16 convert, split DVE/ACT.
                    nc.vector.tensor_scalar_add(
                        ob[:, quarter, :, :DW], ps[:, :, :DW], bsb[:],
                    )
                    nc.scalar.add(
                        ob[:, quarter, :, DW:], ps[:, :, DW:], bsb[:],
                    )

                dst = bass.AP(
                    out_d,
                    idx * M * OBF,
                    [[OBF, M], [1, OBF]],
                )
                nc.sync.dma_start(dst, ob[:])
    nc.finalize()
    return nc


_NC_CACHE = None


def _get_nc():
    global _NC_CACHE
    if _NC_CACHE is None:
        _NC_CACHE = _build_bass()
    return _NC_CACHE


def kernel(x, W, b, trace=False, **trace_kw):
    xin = _build_xin(np.asarray(x, np.float32))
    wt, bias = _build_weights(np.asarray(W, np.float32), np.asarray(b, np.float32))
    zeros = np.zeros((GAP, FW), np.float16)
    in_maps = [
        {"xin": xin[NB * m: NB * (m + 1)], "wt": wt, "bias": bias, "zeros": zeros}
        for m in range(NCORES)
    ]
    res = run_bass_kernel_spmd(
        _get_nc(), in_maps, list(range(NCORES)), trace=trace, **trace_kw
    )
    # Device layout [NB, NCHUNK, 32g+o, (half,t4,w)] -> [B, OUT, H, W]:
    # row = CH*chunk + 4*(4*half + t4) + g
    parts = []
    for m in range(NCORES):
        o = res.results[m]["out"].astype(np.float32)
        o = o.reshape(NB, NCHUNK, G, OUT, 2, T, W_DIM)
        parts.append(o.transpose(0, 3, 1, 4, 5, 2, 6).reshape(NB, OUT, H, W_DIM))
    out = np.ascontiguousarray(np.concatenate(parts, axis=0))
    if trace:
        kernel.last_results = res
    return out
